# revision 2
# baseline (speedup 1.0000x reference)
# Trainium2 Bass kernel for nn_EqPropNetwork (equilibrium-propagation relaxation).
#
# Math (per reference.py):
#   c_h = x @ W1 + b1                                  [B, HID]  (constant over steps)
#   repeat T times:
#     psi = y @ W2.T ; phi = h @ W2
#     h'  = clip(0.5*h + 0.5*c_h + 0.5*psi, 0, 1)
#     y'  = clip(0.25*y + 0.5*phi + 0.5*b2 + 0.25*onehot(target), 0, 1)
#   out = concat(h, y)                                  [B, HID+OUT]
#
# Mapping (per core, B_loc = 4096, pure data parallel over 8 cores):
#   * Feature-major state: partition = feature, free = batch. 4 chunks of 128.
#   * Chunks 0,1 hold h (fp16); their PSUM merge streams 0.5*h AND 0.5*c_h
#     through an identity matmul (no DVE add needed).  Chunks 2,3 hold
#     s = h + c_h; their merge streams 0.5*s only, and a DVE tensor_tensor
#     add (+c_h) rebuilds s after the clip.
#   * u = 0.5*state(+0.5ch) + 0.5*(y @ W2.T) accumulates in PSUM via
#     identity + row-group-packed psi matmuls.  Drain is split: chunks
#     0..2 via ACT relu (PSUM->fp16) + DVE tensor_scalar min (4x mode),
#     chunk 3 via DVE dual-op tensor_scalar clip (1x from PSUM).
#   * y is blocked: partition 32j+i holds y[i, 1024j:1024(j+1)].  The y
#     drive 0.25y + dbar rides diagonal-packed matmuls; dbar folds onehot,
#     b2 and the c_h@W2 correction for the s-state chunks.
#   * All weights/constants are prepared host-side in fp16; outputs return
#     fp16 and are converted on host.
import os
import sys

import numpy as np

if "/opt/trn_rl_repo" not in sys.path:
    sys.path.insert(0, "/opt/trn_rl_repo")

N_CORES = 8
B, IN, HID, OUT = 32768, 784, 512, 10
BLOC = B // N_CORES  # 4096
NBLK = BLOC // 1024  # 4 batch blocks of 1024
KIN = 7              # IN chunks of 112
KC = IN // KIN       # 112
HCH = HID // 128     # 4 hidden chunks
H_CHUNKS = (0, 1)    # h-state chunks
S_CHUNKS = (2, 3)    # s-state chunks (s = h + c_h)

# fp16 const tile column offsets
C_HALFI = 0            # [128, 128] 0.5*I128
C_W2TR = 128           # 4 x [128, 128]: W2Tr_c[32j+i, m] = 0.5*W2[128c+m, i]
C_W2C = 640            # 4 x [128, 10]: W2c[p, 10c+i] = 0.5*W2[128c+p, i]
C_I10Q = 680           # [128, 10] 0.25*I10 at 4 row offsets
C_I10D = 690           # [128, 10] I10 at 4 row offsets
C_DCON = 700           # [128, 1024] blocked 0.25*onehot + 0.5*b2
CF16_W = 1728

_BUILT = {}


def _build(T):
    import concourse.bass as bass
    from concourse import bacc, mybir
    from concourse.tile import TileContext

    f32 = mybir.dt.float32
    f16 = mybir.dt.float16
    Alu = mybir.AluOpType
    Act = mybir.ActivationFunctionType

    nc = bacc.Bacc("TRN2", target_bir_lowering=False)

    xT16 = nc.declare_dram_parameter("xT16", [IN, BLOC], f16, isOutput=False)
    hT16 = nc.declare_dram_parameter("hT16", [HID, BLOC], f16, isOutput=False)
    yB16 = nc.declare_dram_parameter("yB16", [128, 1024], f16, isOutput=False)
    w1h = nc.declare_dram_parameter("w1h", [KC, KIN * HID], f16, isOutput=False)
    b1 = nc.declare_dram_parameter("b1", [HID, 1], f32, isOutput=False)
    cst16 = nc.declare_dram_parameter("cst16", [128, CF16_W], f16, isOutput=False)

    hB_out = nc.declare_dram_parameter("hB_out", [128, HCH * BLOC], f16, isOutput=True)
    yB_out = nc.declare_dram_parameter("yB_out", [128, 1024], f16, isOutput=True)

    with TileContext(nc) as tc:
        with (
            tc.tile_pool(name="const", bufs=1) as constp,
            tc.tile_pool(name="ch", bufs=1) as chp,
            tc.tile_pool(name="state", bufs=2) as sp,
            tc.tile_pool(name="ypool", bufs=2) as yp,
        ):
            cf16 = constp.tile([128, CF16_W], f16, tag="cf16", name="cf16")
            cb1 = constp.tile([128, HCH], f32, tag="cb1", name="cb1")
            dbar = constp.tile([128, 1024], f16, tag="dbar", name="dbar")
            nc.sync.dma_start(out=cf16[:], in_=cst16[:])
            nc.sync.dma_start(
                out=cb1.rearrange("p (c o) -> p c o", c=HCH),
                in_=b1.rearrange("(c p) o -> p c o", c=HCH),
            )

            halfI_t = cf16[:, C_HALFI:C_HALFI + 128]
            W2Tr = [cf16[:, C_W2TR + 128 * c:C_W2TR + 128 * (c + 1)]
                    for c in range(HCH)]
            W2c = [cf16[:, C_W2C + OUT * c:C_W2C + OUT * (c + 1)]
                   for c in range(HCH)]
            I10q_t = cf16[:, C_I10Q:C_I10Q + OUT]
            I10d_t = cf16[:, C_I10D:C_I10D + OUT]
            dcon_t = cf16[:, C_DCON:C_DCON + 1024]
            b1c = [cb1[:, c:c + 1] for c in range(HCH)]

            ch = chp.tile([128, HCH * BLOC], f16, tag="ch", name="ch")
            chv = [ch[:, BLOC * c:BLOC * (c + 1)] for c in range(HCH)]

            # ---------- setup phase A: c_h = x@W1 + b1 ----------
            with (
                tc.tile_pool(name="x16p", bufs=1) as x16p,
                tc.tile_pool(name="spsum", bufs=4, space="PSUM") as spsum,
            ):
                w1_16 = x16p.tile([128, KIN * HID], f16, tag="w1_16", name="w1_16")
                nc.sync.dma_start(out=w1_16[:KC, :], in_=w1h[:, :])
                x16 = x16p.tile([128, KIN * BLOC], f16, tag="x16", name="x16")
                nc.sync.dma_start(
                    out=x16[:KC, :].rearrange("p (k n) -> p k n", k=KIN),
                    in_=xT16.rearrange("(k p) n -> p k n", k=KIN),
                )

                for c in range(HCH):
                    for blk in range(BLOC // 512):
                        ps = spsum.tile([128, 512], f32, tag="spsum", name="spsum")
                        for k in range(KIN):
                            nc.tensor.matmul(
                                ps[:],
                                w1_16[:KC, HID * k + 128 * c:HID * k + 128 * (c + 1)],
                                x16[:KC, BLOC * k + 512 * blk:BLOC * k + 512 * (blk + 1)],
                                start=(k == 0),
                                stop=(k == KIN - 1),
                                tile_position=(0, 0),
                            )
                        nc.scalar.activation(
                            chv[c][:, 512 * blk:512 * (blk + 1)],
                            ps[:],
                            Act.Identity,
                            bias=b1c[c],
                            scale=1.0,
                        )

            # ---------- setup phase B: dbar, y0, state0 ----------
            with (
                tc.tile_pool(name="stage2", bufs=2) as stage2p,
                tc.tile_pool(name="spsum2", bufs=4, space="PSUM") as spsum2,
            ):
                # dbar = dcon - 0.5*(c_h @ W2) restricted to s-state chunks
                ub = stage2p.tile([128, 1024], f16, tag="ub", name="ub", bufs=1)
                nc.vector.memset(ub[:], 0.0)
                for j in range(NBLK):
                    for hf in range(2):
                        ps = spsum2.tile([128, 512], f32, tag="sp2", name="sp2")
                        for ci, c in enumerate(S_CHUNKS):
                            nc.tensor.matmul(
                                ps[32 * j:32 * j + OUT, :],
                                W2c[c],
                                chv[c][:, 1024 * j + 512 * hf:1024 * j + 512 * (hf + 1)],
                                start=(ci == 0),
                                stop=(ci == len(S_CHUNKS) - 1),
                                tile_position=(0, 32 * j),
                            )
                        nc.scalar.activation(
                            ub[32 * j:32 * j + OUT, 512 * hf:512 * (hf + 1)],
                            ps[32 * j:32 * j + OUT, :],
                            Act.Identity,
                            bias=0.0,
                            scale=-1.0,
                        )
                nc.vector.tensor_tensor(dbar[:], ub[:], dcon_t, Alu.add)

                # y0 (blocked, host-prepared)
                ycur = yp.tile([128, 1024], f16, tag="yblk", name="yblk")
                nc.sync.dma_start(out=ycur[:], in_=yB16[:])

                # state0: h chunks direct; s chunks = h0 + c_h
                s0 = sp.tile([128, HCH * BLOC], f16, tag="s", name="s")
                for c in H_CHUNKS:
                    nc.sync.dma_start(
                        out=s0[:, BLOC * c:BLOC * (c + 1)],
                        in_=hT16[128 * c:128 * (c + 1), :],
                    )
                for c in S_CHUNKS:
                    st = stage2p.tile([128, BLOC], f16, tag="h0st", name="h0st")
                    nc.sync.dma_start(out=st[:], in_=hT16[128 * c:128 * (c + 1), :])
                    nc.vector.tensor_tensor(
                        s0[:, BLOC * c:BLOC * (c + 1)], st[:], chv[c][:], Alu.add
                    )
                scur = s0

            # ---------- relaxation loop ----------
            with (
                tc.tile_pool(name="pu", bufs=3, space="PSUM") as pup,
                tc.tile_pool(name="py", bufs=2, space="PSUM") as pyp,
                tc.tile_pool(name="rp", bufs=2) as rp,
                tc.tile_pool(name="mp", bufs=2) as mp,
                tc.tile_pool(name="yr", bufs=2) as yrp,
            ):
                for t in range(T):
                    last = t == T - 1
                    sv = [scur[:, BLOC * c:BLOC * (c + 1)] for c in range(HCH)]
                    # ---- y update ----
                    ynext = yp.tile([128, 1024], f16, tag="yblk", name="yblk")
                    for hf in range(2):
                        sl = slice(512 * hf, 512 * (hf + 1))
                        ps = pyp.tile([128, 512], f32, tag="py", name="py")
                        for c in range(HCH):
                            for j in range(NBLK):
                                nc.tensor.matmul(
                                    ps[32 * j:32 * j + OUT, :],
                                    W2c[c],
                                    sv[c][:, 1024 * j + 512 * hf:
                                          1024 * j + 512 * (hf + 1)],
                                    start=(c == 0),
                                    stop=False,
                                    tile_position=(0, 32 * j),
                                )
                        for j in range(NBLK):
                            nc.tensor.matmul(
                                ps[32 * j:32 * j + OUT, :],
                                I10q_t[32 * j:32 * j + OUT, :],
                                ycur[32 * j:32 * j + OUT, sl],
                                start=False,
                                stop=False,
                                tile_position=(32 * j, 32 * j),
                            )
                        for j in range(NBLK):
                            nc.tensor.matmul(
                                ps[32 * j:32 * j + OUT, :],
                                I10d_t[32 * j:32 * j + OUT, :],
                                dbar[32 * j:32 * j + OUT, sl],
                                start=False,
                                stop=True,
                                tile_position=(32 * j, 32 * j),
                            )
                        yrt = yrp.tile([128, 512], f16, tag="yr", name="yr")
                        nc.scalar.activation(yrt[:], ps[:], Act.Relu)
                        nc.vector.tensor_scalar_min(ynext[:, sl], yrt[:], 1.0)

                    # ---- h update ----
                    snext = sp.tile([128, HCH * BLOC], f16, tag="s", name="s")
                    for c in range(HCH):
                        is_h = c in H_CHUNKS
                        drains = []
                        for rb in range(NBLK):
                            pu = pup.tile([128, 1024], f32, tag="pu", name="pu")
                            for half in range(2):
                                hsl = slice(512 * half, 512 * (half + 1))
                                csl = slice(1024 * rb + 512 * half,
                                            1024 * rb + 512 * (half + 1))
                                nc.tensor.matmul(
                                    pu[:, hsl], halfI_t, sv[c][:, csl],
                                    start=True, stop=False, tile_position=(0, 0),
                                )
                                if is_h:
                                    nc.tensor.matmul(
                                        pu[:, hsl], halfI_t, chv[c][:, csl],
                                        start=False, stop=False,
                                        tile_position=(0, 0),
                                    )
                                nc.tensor.matmul(
                                    pu[:, hsl],
                                    W2Tr[c][32 * rb:32 * rb + OUT, :],
                                    ycur[32 * rb:32 * rb + OUT, hsl],
                                    start=False, stop=True,
                                    tile_position=(32 * rb, 0),
                                )
                            drains.append((pu, rb))
                        ssl = slice(BLOC * c, BLOC * (c + 1))
                        if c != 3:
                            # ACT relu drain into a contiguous r-chunk, then
                            # one 4x-mode min (and +c_h add for s-chunks)
                            rch = rp.tile([128, BLOC], f16, tag="r", name="r")
                            for pu, rb in drains:
                                nc.scalar.activation(
                                    rch[:, 1024 * rb:1024 * (rb + 1)], pu[:],
                                    Act.Relu,
                                )
                            if is_h or last:
                                nc.vector.tensor_scalar_min(
                                    snext[:, ssl], rch[:], 1.0
                                )
                            else:
                                m = mp.tile([128, BLOC], f16, tag="m", name="m")
                                nc.vector.tensor_scalar_min(m[:], rch[:], 1.0)
                                nc.vector.tensor_tensor(
                                    snext[:, ssl], m[:], chv[c][:], Alu.add
                                )
                        else:
                            # DVE dual-op clip drain
                            m = mp.tile([128, BLOC], f16, tag="m", name="m")
                            for pu, rb in drains:
                                nc.vector.tensor_scalar(
                                    m[:, 1024 * rb:1024 * (rb + 1)], pu[:],
                                    0.0, 1.0, Alu.max, Alu.min,
                                )
                            if last:
                                nc.vector.tensor_copy(snext[:, ssl], m[:])
                            else:
                                nc.vector.tensor_tensor(
                                    snext[:, ssl], m[:], chv[c][:], Alu.add
                                )
                    scur = snext
                    ycur = ynext

                # ---------- tail ----------
                for c in range(HCH):
                    nc.sync.dma_start(
                        out=hB_out[:, BLOC * c:BLOC * (c + 1)],
                        in_=scur[:, BLOC * c:BLOC * (c + 1)],
                    )
                nc.sync.dma_start(out=yB_out[:], in_=ycur[:])

    if not nc.is_finalized():
        nc.finalize()
    return nc


def _consts(W2, b2):
    cst16 = np.zeros((128, CF16_W), dtype=np.float16)
    cst16[:, C_HALFI:C_HALFI + 128] = 0.5 * np.eye(128, dtype=np.float16)
    W2h = 0.5 * W2.astype(np.float32)
    for c in range(HCH):
        # W2Tr_c[32j+i, m] = 0.5*W2[128c+m, i]
        blkT = W2h[128 * c:128 * (c + 1), :].T.astype(np.float16)  # [10, 128]
        for j in range(NBLK):
            cst16[32 * j:32 * j + OUT, C_W2TR + 128 * c:C_W2TR + 128 * (c + 1)] = blkT
        # W2c[p, 10c+i] = 0.5*W2[128c+p, i]
        cst16[:, C_W2C + OUT * c:C_W2C + OUT * (c + 1)] = W2h[
            128 * c:128 * (c + 1), :
        ].astype(np.float16)
    for j in range(NBLK):
        for i in range(OUT):
            cst16[32 * j + i, C_I10Q + i] = 0.25
            cst16[32 * j + i, C_I10D + i] = 1.0
    return cst16


def kernel(**inputs):
    from concourse import bass_utils

    x = np.asarray(inputs["x"], dtype=np.float32)
    h0 = np.asarray(inputs["h_init"], dtype=np.float32)
    y0 = np.asarray(inputs["y_init"], dtype=np.float32)
    W1 = np.asarray(inputs["W1"], dtype=np.float32)
    W2 = np.asarray(inputs["W2"], dtype=np.float32)
    b1 = np.ascontiguousarray(
        np.asarray(inputs["b1"], dtype=np.float32).reshape(HID, 1)
    )
    b2 = np.asarray(inputs["b2"], dtype=np.float32).reshape(OUT)
    target = np.asarray(inputs["target"]).astype(np.int64)
    T = int(inputs["T"])

    xT16 = np.ascontiguousarray(x.T.astype(np.float16))      # [IN, B]
    hT16 = np.ascontiguousarray(h0.T.astype(np.float16))     # [HID, B]
    # w1h[p, 512k+f] = W1[112k+p, f]
    w1h = np.ascontiguousarray(
        W1.reshape(KIN, KC, HID).transpose(1, 0, 2).reshape(KC, KIN * HID)
    ).astype(np.float16)

    cst16 = _consts(W2, b2)
    # dcon blocked: [32j+i, n] = 0.25*onehot(target[bloc*k+1024j+n] == i) + 0.5*b2[i]
    # (core-dependent -> built per core below)

    key = T
    if key not in _BUILT:
        _BUILT[key] = _build(T)
    nc = _BUILT[key]

    in_maps = []
    for k in range(N_CORES):
        sl = slice(k * BLOC, (k + 1) * BLOC)
        tgt_k = target[sl]
        y0_k = y0[sl]  # [BLOC, OUT]
        yB = np.zeros((128, 1024), dtype=np.float16)
        dcon = np.zeros((128, 1024), dtype=np.float16)
        for j in range(NBLK):
            seg = slice(1024 * j, 1024 * (j + 1))
            yB[32 * j:32 * j + OUT, :] = y0_k[seg].T
            oh = (tgt_k[seg][None, :] == np.arange(OUT)[:, None])
            dcon[32 * j:32 * j + OUT, :] = (
                0.25 * oh + 0.5 * b2[:, None]
            ).astype(np.float16)
        ck = cst16.copy()
        ck[:, C_DCON:C_DCON + 1024] = dcon
        in_maps.append({
            "xT16": np.ascontiguousarray(xT16[:, sl]),
            "hT16": np.ascontiguousarray(hT16[:, sl]),
            "yB16": yB,
            "w1h": w1h,
            "b1": b1,
            "cst16": ck,
        })

    res = bass_utils.run_bass_kernel_spmd(nc, in_maps, list(range(N_CORES)))
    globals()["_LAST_RESULTS"] = res

    out = np.empty((B, HID + OUT), dtype=np.float32)
    for k in range(N_CORES):
        sl = slice(k * BLOC, (k + 1) * BLOC)
        hB = np.asarray(res.results[k]["hB_out"])  # [128, 4*4096] f16
        hT = hB.reshape(128, HCH, BLOC).transpose(1, 0, 2).reshape(HID, BLOC)
        out[sl, :HID] = hT.T.astype(np.float32)
        yB = np.asarray(res.results[k]["yB_out"])  # [128, 1024] f16
        yblk = yB.reshape(NBLK, 32, 1024)[:, :OUT, :]  # [4, 10, 1024]
        out[sl, HID:] = yblk.transpose(0, 2, 1).reshape(BLOC, OUT).astype(np.float32)
    return out


# revision 4
# speedup vs baseline: 1.2030x; 1.2030x over previous
# Trainium2 Bass kernel for nn_EqPropNetwork (equilibrium-propagation relaxation).
#
# Math (per reference.py):
#   c_h = x @ W1 + b1                                  [B, HID]  (constant)
#   repeat T times:
#     psi = y @ W2.T ; phi = h @ W2
#     h'  = clip(0.5*h + 0.5*c_h + 0.5*psi, 0, 1)
#     y'  = clip(0.25*y + 0.5*phi + 0.5*b2 + 0.25*onehot(target), 0, 1)
#   out = concat(h, y)                                  [B, HID+OUT]
#
# Mapping (per core, B_loc = 4096, data parallel over 8 cores):
#   * Feature-major state s = h + c_h (fp16), 4 chunks of 128 features.
#   * Chunks 0..2: PSUM merge u = 0.5*s + 0.5*psi via identity matmuls
#     (grouped back-to-back so LDWEIGHTS hides) + row-group-packed psi;
#     drain via ACT relu (fp16), then one DVE tensor_scalar min (4x mode)
#     and one tensor_tensor add (+c_h, 2x mode) per chunk.
#   * Chunk 3: psi-only PSUM; DVE scalar_tensor_tensor merges 0.5*s + pu,
#     then 4x-mode clip and the +c_h add.  Keeps the PE off this chunk.
#   * y blocked: partition 32j+i holds y[i, 1024j:1024(j+1)]; drive
#     0.25y+dbar rides diagonal-packed matmuls; dbar folds onehot, b2 and
#     the c_h@W2 correction (all chunks use s).  ACT relu + DVE min drain.
#   * All weights/constants prepared host-side in fp16; outputs fp16.
import os
import sys

import numpy as np

if "/opt/trn_rl_repo" not in sys.path:
    sys.path.insert(0, "/opt/trn_rl_repo")

N_CORES = 8
B, IN, HID, OUT = 32768, 784, 512, 10
BLOC = B // N_CORES  # 4096
NBLK = BLOC // 1024  # 4 batch blocks of 1024
KIN = 7              # IN chunks of 112
KC = IN // KIN       # 112
HCH = HID // 128     # 4 hidden chunks
DVE_CHUNK = 3        # chunk merged on DVE instead of PE

# fp16 const tile column offsets
C_HALFI = 0            # [128, 128] 0.5*I128
C_W2TR = 128           # 4 x [128, 128]: W2Tr_c[32j+i, m] = 0.5*W2[128c+m, i]
C_W2C = 640            # 4 x [128, 10]: W2c[p, 10c+i] = 0.5*W2[128c+p, i]
C_I10Q = 680           # [128, 10] 0.25*I10 at 4 row offsets
C_I10D = 690           # [128, 10] I10 at 4 row offsets
C_DCON = 700           # [128, 1024] blocked 0.25*onehot + 0.5*b2
CF16_W = 1728

_BUILT = {}


def _build(T):
    import concourse.bass as bass
    from concourse import bacc, mybir
    from concourse.tile import TileContext

    f32 = mybir.dt.float32
    f16 = mybir.dt.float16
    Alu = mybir.AluOpType
    Act = mybir.ActivationFunctionType

    nc = bacc.Bacc("TRN2", target_bir_lowering=False)

    xT16 = nc.declare_dram_parameter("xT16", [IN, BLOC], f16, isOutput=False)
    hT16 = nc.declare_dram_parameter("hT16", [HID, BLOC], f16, isOutput=False)
    yB16 = nc.declare_dram_parameter("yB16", [128, 1024], f16, isOutput=False)
    w1h = nc.declare_dram_parameter("w1h", [KC, KIN * HID], f16, isOutput=False)
    b1 = nc.declare_dram_parameter("b1", [HID, 1], f32, isOutput=False)
    cst16 = nc.declare_dram_parameter("cst16", [128, CF16_W], f16, isOutput=False)

    hB_out = nc.declare_dram_parameter("hB_out", [128, HCH * BLOC], f16, isOutput=True)
    yB_out = nc.declare_dram_parameter("yB_out", [128, 1024], f16, isOutput=True)

    with TileContext(nc) as tc:
        with (
            tc.tile_pool(name="const", bufs=1) as constp,
            tc.tile_pool(name="ch", bufs=1) as chp,
            tc.tile_pool(name="state", bufs=2) as sp,
            tc.tile_pool(name="ypool", bufs=2) as yp,
        ):
            cf16 = constp.tile([128, CF16_W], f16, tag="cf16", name="cf16")
            cb1 = constp.tile([128, HCH], f32, tag="cb1", name="cb1")
            dbar = constp.tile([128, 1024], f16, tag="dbar", name="dbar")
            nc.sync.dma_start(out=cf16[:], in_=cst16[:])
            nc.sync.dma_start(
                out=cb1.rearrange("p (c o) -> p c o", c=HCH),
                in_=b1.rearrange("(c p) o -> p c o", c=HCH),
            )

            halfI_t = cf16[:, C_HALFI:C_HALFI + 128]
            W2Tr = [cf16[:, C_W2TR + 128 * c:C_W2TR + 128 * (c + 1)]
                    for c in range(HCH)]
            W2c = [cf16[:, C_W2C + OUT * c:C_W2C + OUT * (c + 1)]
                   for c in range(HCH)]
            I10q_t = cf16[:, C_I10Q:C_I10Q + OUT]
            I10d_t = cf16[:, C_I10D:C_I10D + OUT]
            dcon_t = cf16[:, C_DCON:C_DCON + 1024]
            b1c = [cb1[:, c:c + 1] for c in range(HCH)]

            ch = chp.tile([128, HCH * BLOC], f16, tag="ch", name="ch")
            chv = [ch[:, BLOC * c:BLOC * (c + 1)] for c in range(HCH)]

            # ---------- setup phase A: c_h = x@W1 + b1 ----------
            with (
                tc.tile_pool(name="x16p", bufs=1) as x16p,
                tc.tile_pool(name="spsum", bufs=4, space="PSUM") as spsum,
            ):
                w1_16 = x16p.tile([128, KIN * HID], f16, tag="w1_16", name="w1_16")
                nc.sync.dma_start(out=w1_16[:KC, :], in_=w1h[:, :])
                x16 = x16p.tile([128, KIN * BLOC], f16, tag="x16", name="x16")
                nc.sync.dma_start(
                    out=x16[:KC, :].rearrange("p (k n) -> p k n", k=KIN),
                    in_=xT16.rearrange("(k p) n -> p k n", k=KIN),
                )

                for c in range(HCH):
                    for blk in range(BLOC // 512):
                        ps = spsum.tile([128, 512], f32, tag="spsum", name="spsum")
                        for k in range(KIN):
                            nc.tensor.matmul(
                                ps[:],
                                w1_16[:KC, HID * k + 128 * c:HID * k + 128 * (c + 1)],
                                x16[:KC, BLOC * k + 512 * blk:BLOC * k + 512 * (blk + 1)],
                                start=(k == 0),
                                stop=(k == KIN - 1),
                                tile_position=(0, 0),
                            )
                        nc.scalar.activation(
                            chv[c][:, 512 * blk:512 * (blk + 1)],
                            ps[:],
                            Act.Identity,
                            bias=b1c[c],
                            scale=1.0,
                        )

            # ---------- setup phase B: dbar, y0, state0 ----------
            with (
                tc.tile_pool(name="stage2", bufs=2) as stage2p,
                tc.tile_pool(name="spsum2", bufs=4, space="PSUM") as spsum2,
            ):
                # dbar = dcon - 0.5*(c_h @ W2)  (all chunks use s-state)
                ub = stage2p.tile([128, 1024], f16, tag="ub", name="ub", bufs=1)
                nc.vector.memset(ub[:], 0.0)
                for j in range(NBLK):
                    for hf in range(2):
                        ps = spsum2.tile([128, 512], f32, tag="sp2", name="sp2")
                        for c in range(HCH):
                            nc.tensor.matmul(
                                ps[32 * j:32 * j + OUT, :],
                                W2c[c],
                                chv[c][:, 1024 * j + 512 * hf:1024 * j + 512 * (hf + 1)],
                                start=(c == 0),
                                stop=(c == HCH - 1),
                                tile_position=(0, 32 * j),
                            )
                        nc.scalar.activation(
                            ub[32 * j:32 * j + OUT, 512 * hf:512 * (hf + 1)],
                            ps[32 * j:32 * j + OUT, :],
                            Act.Identity,
                            bias=0.0,
                            scale=-1.0,
                        )
                nc.vector.tensor_tensor(dbar[:], ub[:], dcon_t, Alu.add)

                # y0 (blocked, host-prepared)
                ycur = yp.tile([128, 1024], f16, tag="yblk", name="yblk")
                nc.sync.dma_start(out=ycur[:], in_=yB16[:])

                # state0 = h0 + c_h
                s0 = sp.tile([128, HCH * BLOC], f16, tag="s", name="s")
                for c in range(HCH):
                    st = stage2p.tile([128, BLOC], f16, tag="h0st", name="h0st")
                    nc.sync.dma_start(out=st[:], in_=hT16[128 * c:128 * (c + 1), :])
                    nc.vector.tensor_tensor(
                        s0[:, BLOC * c:BLOC * (c + 1)], st[:], chv[c][:], Alu.add
                    )
                scur = s0

            # ---------- relaxation loop ----------
            with (
                tc.tile_pool(name="pu", bufs=3, space="PSUM") as pup,
                tc.tile_pool(name="py", bufs=2, space="PSUM") as pyp,
                tc.tile_pool(name="rp", bufs=2) as rp,
                tc.tile_pool(name="mp", bufs=2) as mp,
                tc.tile_pool(name="yr", bufs=2) as yrp,
            ):
                for t in range(T):
                    last = t == T - 1
                    sv = [scur[:, BLOC * c:BLOC * (c + 1)] for c in range(HCH)]
                    # ---- y update ----
                    ynext = yp.tile([128, 1024], f16, tag="yblk", name="yblk")
                    for hf in range(2):
                        sl = slice(512 * hf, 512 * (hf + 1))
                        ps = pyp.tile([128, 512], f32, tag="py", name="py")
                        for c in range(HCH):
                            for j in range(NBLK):
                                nc.tensor.matmul(
                                    ps[32 * j:32 * j + OUT, :],
                                    W2c[c],
                                    sv[c][:, 1024 * j + 512 * hf:
                                          1024 * j + 512 * (hf + 1)],
                                    start=(c == 0),
                                    stop=False,
                                    tile_position=(0, 32 * j),
                                )
                        for j in range(NBLK):
                            nc.tensor.matmul(
                                ps[32 * j:32 * j + OUT, :],
                                I10q_t[32 * j:32 * j + OUT, :],
                                ycur[32 * j:32 * j + OUT, sl],
                                start=False,
                                stop=False,
                                tile_position=(32 * j, 32 * j),
                            )
                        for j in range(NBLK):
                            nc.tensor.matmul(
                                ps[32 * j:32 * j + OUT, :],
                                I10d_t[32 * j:32 * j + OUT, :],
                                dbar[32 * j:32 * j + OUT, sl],
                                start=False,
                                stop=True,
                                tile_position=(32 * j, 32 * j),
                            )
                        yrt = yrp.tile([128, 512], f16, tag="yr", name="yr")
                        nc.scalar.activation(yrt[:], ps[:], Act.Relu)
                        nc.vector.tensor_scalar_min(ynext[:, sl], yrt[:], 1.0)

                    # ---- h update ----
                    snext = sp.tile([128, HCH * BLOC], f16, tag="s", name="s")
                    for c in range(HCH):
                        ssl = slice(BLOC * c, BLOC * (c + 1))
                        if c != DVE_CHUNK:
                            # PE-merged: ids grouped in rb-pairs (LDW hides),
                            # then row-packed psi pairs
                            pus = []
                            for pair in range(2):
                                ptiles = []
                                for rb2 in range(2):
                                    rb = 2 * pair + rb2
                                    pu = pup.tile([128, 1024], f32, tag="pu",
                                                  name="pu")
                                    for half in range(2):
                                        hsl = slice(512 * half, 512 * (half + 1))
                                        csl = slice(1024 * rb + 512 * half,
                                                    1024 * rb + 512 * (half + 1))
                                        nc.tensor.matmul(
                                            pu[:, hsl], halfI_t, sv[c][:, csl],
                                            start=True, stop=False,
                                            tile_position=(0, 0),
                                        )
                                    ptiles.append((pu, rb))
                                for pu, rb in ptiles:
                                    for half in range(2):
                                        hsl = slice(512 * half, 512 * (half + 1))
                                        nc.tensor.matmul(
                                            pu[:, hsl],
                                            W2Tr[c][32 * rb:32 * rb + OUT, :],
                                            ycur[32 * rb:32 * rb + OUT, hsl],
                                            start=False, stop=True,
                                            tile_position=(32 * rb, 0),
                                        )
                                pus.extend(ptiles)
                            rch = rp.tile([128, BLOC], f16, tag="r", name="r")
                            for pu, rb in pus:
                                nc.scalar.activation(
                                    rch[:, 1024 * rb:1024 * (rb + 1)], pu[:],
                                    Act.Relu,
                                )
                            if last:
                                nc.vector.tensor_scalar_min(
                                    snext[:, ssl], rch[:], 1.0
                                )
                            else:
                                m = mp.tile([128, BLOC], f16, tag="m", name="m")
                                nc.vector.tensor_scalar_min(m[:], rch[:], 1.0)
                                nc.vector.tensor_tensor(
                                    snext[:, ssl], m[:], chv[c][:], Alu.add
                                )
                        else:
                            # DVE-merged: psi-only PSUM, stt merge, clip, add
                            pus = []
                            for rb in range(NBLK):
                                pu = pup.tile([128, 1024], f32, tag="pu", name="pu")
                                for half in range(2):
                                    hsl = slice(512 * half, 512 * (half + 1))
                                    nc.tensor.matmul(
                                        pu[:, hsl],
                                        W2Tr[c][32 * rb:32 * rb + OUT, :],
                                        ycur[32 * rb:32 * rb + OUT, hsl],
                                        start=True,
                                        stop=True,
                                        tile_position=(32 * rb, 0),
                                    )
                                pus.append((pu, rb))
                            u3 = rp.tile([128, BLOC], f16, tag="r", name="r")
                            for pu, rb in pus:
                                nc.vector.scalar_tensor_tensor(
                                    u3[:, 1024 * rb:1024 * (rb + 1)],
                                    sv[c][:, 1024 * rb:1024 * (rb + 1)],
                                    0.5, pu[:], Alu.mult, Alu.add,
                                )
                            if last:
                                nc.vector.tensor_scalar(
                                    snext[:, ssl], u3[:], 0.0, 1.0,
                                    Alu.max, Alu.min,
                                )
                            else:
                                m = mp.tile([128, BLOC], f16, tag="m", name="m")
                                nc.vector.tensor_scalar(
                                    m[:], u3[:], 0.0, 1.0, Alu.max, Alu.min
                                )
                                nc.vector.tensor_tensor(
                                    snext[:, ssl], m[:], chv[c][:], Alu.add
                                )
                    scur = snext
                    ycur = ynext

                # ---------- tail ----------
                for c in range(HCH):
                    nc.sync.dma_start(
                        out=hB_out[:, BLOC * c:BLOC * (c + 1)],
                        in_=scur[:, BLOC * c:BLOC * (c + 1)],
                    )
                nc.sync.dma_start(out=yB_out[:], in_=ycur[:])

    if not nc.is_finalized():
        nc.finalize()
    return nc


def _consts(W2, b2):
    cst16 = np.zeros((128, CF16_W), dtype=np.float16)
    cst16[:, C_HALFI:C_HALFI + 128] = 0.5 * np.eye(128, dtype=np.float16)
    W2h = 0.5 * W2.astype(np.float32)
    for c in range(HCH):
        blkT = W2h[128 * c:128 * (c + 1), :].T.astype(np.float16)  # [10, 128]
        for j in range(NBLK):
            cst16[32 * j:32 * j + OUT, C_W2TR + 128 * c:C_W2TR + 128 * (c + 1)] = blkT
        cst16[:, C_W2C + OUT * c:C_W2C + OUT * (c + 1)] = W2h[
            128 * c:128 * (c + 1), :
        ].astype(np.float16)
    for j in range(NBLK):
        for i in range(OUT):
            cst16[32 * j + i, C_I10Q + i] = 0.25
            cst16[32 * j + i, C_I10D + i] = 1.0
    return cst16


def kernel(**inputs):
    from concourse import bass_utils

    x = np.asarray(inputs["x"], dtype=np.float32)
    h0 = np.asarray(inputs["h_init"], dtype=np.float32)
    y0 = np.asarray(inputs["y_init"], dtype=np.float32)
    W1 = np.asarray(inputs["W1"], dtype=np.float32)
    W2 = np.asarray(inputs["W2"], dtype=np.float32)
    b1 = np.ascontiguousarray(
        np.asarray(inputs["b1"], dtype=np.float32).reshape(HID, 1)
    )
    b2 = np.asarray(inputs["b2"], dtype=np.float32).reshape(OUT)
    target = np.asarray(inputs["target"]).astype(np.int64)
    T = int(inputs["T"])

    xT16 = np.ascontiguousarray(x.T.astype(np.float16))      # [IN, B]
    hT16 = np.ascontiguousarray(h0.T.astype(np.float16))     # [HID, B]
    w1h = np.ascontiguousarray(
        W1.reshape(KIN, KC, HID).transpose(1, 0, 2).reshape(KC, KIN * HID)
    ).astype(np.float16)

    cst16 = _consts(W2, b2)

    key = T
    if key not in _BUILT:
        _BUILT[key] = _build(T)
    nc = _BUILT[key]

    in_maps = []
    for k in range(N_CORES):
        sl = slice(k * BLOC, (k + 1) * BLOC)
        tgt_k = target[sl]
        y0_k = y0[sl]  # [BLOC, OUT]
        yB = np.zeros((128, 1024), dtype=np.float16)
        dcon = np.zeros((128, 1024), dtype=np.float16)
        for j in range(NBLK):
            seg = slice(1024 * j, 1024 * (j + 1))
            yB[32 * j:32 * j + OUT, :] = y0_k[seg].T
            oh = (tgt_k[seg][None, :] == np.arange(OUT)[:, None])
            dcon[32 * j:32 * j + OUT, :] = (
                0.25 * oh + 0.5 * b2[:, None]
            ).astype(np.float16)
        ck = cst16.copy()
        ck[:, C_DCON:C_DCON + 1024] = dcon
        in_maps.append({
            "xT16": np.ascontiguousarray(xT16[:, sl]),
            "hT16": np.ascontiguousarray(hT16[:, sl]),
            "yB16": yB,
            "w1h": w1h,
            "b1": b1,
            "cst16": ck,
        })

    res = bass_utils.run_bass_kernel_spmd(nc, in_maps, list(range(N_CORES)))
    globals()["_LAST_RESULTS"] = res

    out = np.empty((B, HID + OUT), dtype=np.float32)
    for k in range(N_CORES):
        sl = slice(k * BLOC, (k + 1) * BLOC)
        hB = np.asarray(res.results[k]["hB_out"])  # [128, 4*4096] f16
        hT = hB.reshape(128, HCH, BLOC).transpose(1, 0, 2).reshape(HID, BLOC)
        out[sl, :HID] = hT.T.astype(np.float32)
        yB = np.asarray(res.results[k]["yB_out"])  # [128, 1024] f16
        yblk = yB.reshape(NBLK, 32, 1024)[:, :OUT, :]  # [4, 10, 1024]
        out[sl, HID:] = yblk.transpose(0, 2, 1).reshape(BLOC, OUT).astype(np.float32)
    return out


# revision 5
# speedup vs baseline: 1.3396x; 1.1136x over previous
# Trainium2 Bass kernel for nn_EqPropNetwork (equilibrium-propagation relaxation).
#
# Math (per reference.py):
#   c_h = x @ W1 + b1                                  [B, HID]  (constant)
#   repeat T times:
#     psi = y @ W2.T ; phi = h @ W2
#     h'  = clip(0.5*h + 0.5*c_h + 0.5*psi, 0, 1)
#     y'  = clip(0.25*y + 0.5*phi + 0.5*b2 + 0.25*onehot(target), 0, 1)
#   out = concat(h, y)                                  [B, HID+OUT]
#
# Mapping (per core, B_loc = 4096, data parallel over 8 cores):
#   * Feature-major state s = h + c_h (fp16), 4 chunks of 128 features.
#   * PSUM in 1-bank [128, 512] tiles (7 bufs + 1 y buf) so psi matmuls can
#     row-group-pack 4-wide across batch blocks.
#   * Chunks 0..2: PE merge u = 0.5*s + 0.5*psi (identity matmuls grouped
#     back-to-back so LDWEIGHTS hides, then a 4-wide psi quad); ACT relu
#     drain, DVE 4x-mode min, +c_h add on DVE (chunk 0's add on GPSIMD).
#   * Chunk 3: psi-only PSUM; DVE scalar_tensor_tensor merge, 4x clip, add.
#   * y blocked: partition 32j+i holds y[i, 1024j:1024(j+1)]; drive
#     0.25y+dbar rides diagonal-packed matmuls; dbar folds onehot, b2 and
#     the c_h@W2 correction.  ACT relu + DVE min drain.
#   * All weights/constants prepared host-side in fp16; outputs fp16.
import os
import sys

import numpy as np

if "/opt/trn_rl_repo" not in sys.path:
    sys.path.insert(0, "/opt/trn_rl_repo")

N_CORES = 8
B, IN, HID, OUT = 32768, 784, 512, 10
BLOC = B // N_CORES  # 4096
NBLK = BLOC // 1024  # 4 batch blocks of 1024
KIN = 7              # IN chunks of 112
KC = IN // KIN       # 112
HCH = HID // 128     # 4 hidden chunks
DVE_CHUNK = 3        # chunk merged on DVE instead of PE
GP_ADD_CHUNKS = (0,)  # chunks whose +c_h add runs on GPSIMD

# fp16 const tile column offsets
C_HALFI = 0            # [128, 128] 0.5*I128
C_W2TR = 128           # 4 x [128, 128]: W2Tr_c[32j+i, m] = 0.5*W2[128c+m, i]
C_W2C = 640            # 4 x [128, 10]: W2c[p, 10c+i] = 0.5*W2[128c+p, i]
C_I10Q = 680           # [128, 10] 0.25*I10 at 4 row offsets
C_I10D = 690           # [128, 10] I10 at 4 row offsets
C_DCON = 700           # [128, 1024] blocked 0.25*onehot + 0.5*b2
CF16_W = 1728

_BUILT = {}


def _build(T):
    import concourse.bass as bass
    from concourse import bacc, mybir
    from concourse.tile import TileContext

    f32 = mybir.dt.float32
    f16 = mybir.dt.float16
    Alu = mybir.AluOpType
    Act = mybir.ActivationFunctionType

    nc = bacc.Bacc("TRN2", target_bir_lowering=False)

    xT16 = nc.declare_dram_parameter("xT16", [IN, BLOC], f16, isOutput=False)
    hT16 = nc.declare_dram_parameter("hT16", [HID, BLOC], f16, isOutput=False)
    yB16 = nc.declare_dram_parameter("yB16", [128, 1024], f16, isOutput=False)
    w1h = nc.declare_dram_parameter("w1h", [KC, KIN * HID], f16, isOutput=False)
    b1 = nc.declare_dram_parameter("b1", [HID, 1], f32, isOutput=False)
    cst16 = nc.declare_dram_parameter("cst16", [128, CF16_W], f16, isOutput=False)

    hB_out = nc.declare_dram_parameter("hB_out", [128, HCH * BLOC], f16, isOutput=True)
    yB_out = nc.declare_dram_parameter("yB_out", [128, 1024], f16, isOutput=True)

    with TileContext(nc) as tc:
        with (
            tc.tile_pool(name="const", bufs=1) as constp,
            tc.tile_pool(name="ch", bufs=1) as chp,
            tc.tile_pool(name="state", bufs=2) as sp,
            tc.tile_pool(name="ypool", bufs=2) as yp,
        ):
            cf16 = constp.tile([128, CF16_W], f16, tag="cf16", name="cf16")
            cb1 = constp.tile([128, HCH], f32, tag="cb1", name="cb1")
            dbar = constp.tile([128, 1024], f16, tag="dbar", name="dbar")
            nc.sync.dma_start(out=cf16[:], in_=cst16[:])
            nc.sync.dma_start(
                out=cb1.rearrange("p (c o) -> p c o", c=HCH),
                in_=b1.rearrange("(c p) o -> p c o", c=HCH),
            )

            halfI_t = cf16[:, C_HALFI:C_HALFI + 128]
            W2Tr = [cf16[:, C_W2TR + 128 * c:C_W2TR + 128 * (c + 1)]
                    for c in range(HCH)]
            W2c = [cf16[:, C_W2C + OUT * c:C_W2C + OUT * (c + 1)]
                   for c in range(HCH)]
            I10q_t = cf16[:, C_I10Q:C_I10Q + OUT]
            I10d_t = cf16[:, C_I10D:C_I10D + OUT]
            dcon_t = cf16[:, C_DCON:C_DCON + 1024]
            b1c = [cb1[:, c:c + 1] for c in range(HCH)]

            ch = chp.tile([128, HCH * BLOC], f16, tag="ch", name="ch")
            chv = [ch[:, BLOC * c:BLOC * (c + 1)] for c in range(HCH)]

            # ---------- setup phase A: c_h = x@W1 + b1 ----------
            with (
                tc.tile_pool(name="x16p", bufs=1) as x16p,
                tc.tile_pool(name="spsum", bufs=4, space="PSUM") as spsum,
            ):
                w1_16 = x16p.tile([128, KIN * HID], f16, tag="w1_16", name="w1_16")
                nc.sync.dma_start(out=w1_16[:KC, :], in_=w1h[:, :])
                x16 = x16p.tile([128, KIN * BLOC], f16, tag="x16", name="x16")
                nc.sync.dma_start(
                    out=x16[:KC, :].rearrange("p (k n) -> p k n", k=KIN),
                    in_=xT16.rearrange("(k p) n -> p k n", k=KIN),
                )

                for c in range(HCH):
                    for blk in range(BLOC // 512):
                        ps = spsum.tile([128, 512], f32, tag="spsum", name="spsum")
                        for k in range(KIN):
                            nc.tensor.matmul(
                                ps[:],
                                w1_16[:KC, HID * k + 128 * c:HID * k + 128 * (c + 1)],
                                x16[:KC, BLOC * k + 512 * blk:BLOC * k + 512 * (blk + 1)],
                                start=(k == 0),
                                stop=(k == KIN - 1),
                                tile_position=(0, 0),
                            )
                        nc.scalar.activation(
                            chv[c][:, 512 * blk:512 * (blk + 1)],
                            ps[:],
                            Act.Identity,
                            bias=b1c[c],
                            scale=1.0,
                        )

            # ---------- setup phase B: dbar, y0, state0 ----------
            with (
                tc.tile_pool(name="stage2", bufs=2) as stage2p,
                tc.tile_pool(name="spsum2", bufs=4, space="PSUM") as spsum2,
            ):
                # dbar = dcon - 0.5*(c_h @ W2)
                ub = stage2p.tile([128, 1024], f16, tag="ub", name="ub", bufs=1)
                nc.vector.memset(ub[:], 0.0)
                for j in range(NBLK):
                    for hf in range(2):
                        ps = spsum2.tile([128, 512], f32, tag="sp2", name="sp2")
                        for c in range(HCH):
                            nc.tensor.matmul(
                                ps[32 * j:32 * j + OUT, :],
                                W2c[c],
                                chv[c][:, 1024 * j + 512 * hf:1024 * j + 512 * (hf + 1)],
                                start=(c == 0),
                                stop=(c == HCH - 1),
                                tile_position=(0, 32 * j),
                            )
                        nc.scalar.activation(
                            ub[32 * j:32 * j + OUT, 512 * hf:512 * (hf + 1)],
                            ps[32 * j:32 * j + OUT, :],
                            Act.Identity,
                            bias=0.0,
                            scale=-1.0,
                        )
                nc.vector.tensor_tensor(dbar[:], ub[:], dcon_t, Alu.add)

                # y0 (blocked, host-prepared)
                ycur = yp.tile([128, 1024], f16, tag="yblk", name="yblk")
                nc.sync.dma_start(out=ycur[:], in_=yB16[:])

                # state0 = h0 + c_h
                s0 = sp.tile([128, HCH * BLOC], f16, tag="s", name="s")
                for c in range(HCH):
                    st = stage2p.tile([128, BLOC], f16, tag="h0st", name="h0st")
                    nc.sync.dma_start(out=st[:], in_=hT16[128 * c:128 * (c + 1), :])
                    nc.vector.tensor_tensor(
                        s0[:, BLOC * c:BLOC * (c + 1)], st[:], chv[c][:], Alu.add
                    )
                scur = s0

            # ---------- relaxation loop ----------
            with (
                tc.tile_pool(name="pu", bufs=7, space="PSUM") as pup,
                tc.tile_pool(name="py", bufs=1, space="PSUM") as pyp,
                tc.tile_pool(name="rp", bufs=3) as rp,
                tc.tile_pool(name="mp", bufs=3) as mp,
                tc.tile_pool(name="yr", bufs=2) as yrp,
            ):
                for t in range(T):
                    last = t == T - 1
                    sv = [scur[:, BLOC * c:BLOC * (c + 1)] for c in range(HCH)]
                    snext = sp.tile([128, HCH * BLOC], f16, tag="s", name="s")

                    def finish_chunk(c, rch, on_gp):
                        ssl = slice(BLOC * c, BLOC * (c + 1))
                        if last:
                            nc.vector.tensor_scalar_min(snext[:, ssl], rch[:], 1.0)
                        else:
                            m = mp.tile([128, BLOC], f16, tag="m", name="m")
                            nc.vector.tensor_scalar_min(m[:], rch[:], 1.0)
                            eng = nc.gpsimd if on_gp else nc.vector
                            for half in range(2):
                                hs = slice(2048 * half, 2048 * (half + 1))
                                cs = slice(BLOC * c + 2048 * half,
                                           BLOC * c + 2048 * (half + 1))
                                eng.tensor_tensor(
                                    snext[:, cs], m[:, hs],
                                    chv[c][:, hs], Alu.add,
                                )

                    # ---- chunk 3 (DVE-merged) first so DVE starts early ----
                    c = DVE_CHUNK
                    u3 = rp.tile([128, BLOC], f16, tag="r", name="r")
                    for half in range(2):
                        pts = []
                        for rb in range(NBLK):
                            pu = pup.tile([128, 512], f32, tag="pu", name="pu")
                            nc.tensor.matmul(
                                pu[:],
                                W2Tr[c][32 * rb:32 * rb + OUT, :],
                                ycur[32 * rb:32 * rb + OUT,
                                     512 * half:512 * (half + 1)],
                                start=True, stop=True,
                                tile_position=(32 * rb, 0),
                            )
                            pts.append((pu, rb))
                        for pu, rb in pts:
                            nc.vector.scalar_tensor_tensor(
                                u3[:, 1024 * rb + 512 * half:
                                   1024 * rb + 512 * (half + 1)],
                                sv[c][:, 1024 * rb + 512 * half:
                                      1024 * rb + 512 * (half + 1)],
                                0.5, pu[:], Alu.mult, Alu.add,
                            )
                    ssl3 = slice(BLOC * c, BLOC * (c + 1))
                    if last:
                        nc.vector.tensor_scalar(
                            snext[:, ssl3], u3[:], 0.0, 1.0, Alu.max, Alu.min
                        )
                    else:
                        m3 = mp.tile([128, BLOC], f16, tag="m", name="m")
                        nc.vector.tensor_scalar(
                            m3[:], u3[:], 0.0, 1.0, Alu.max, Alu.min
                        )
                        nc.vector.tensor_tensor(
                            snext[:, ssl3], m3[:], chv[c][:], Alu.add
                        )

                    # ---- y update ----
                    ynext = yp.tile([128, 1024], f16, tag="yblk", name="yblk")
                    for hf in range(2):
                        sl = slice(512 * hf, 512 * (hf + 1))
                        ps = pyp.tile([128, 512], f32, tag="py", name="py")
                        for c2 in range(HCH):
                            for j in range(NBLK):
                                nc.tensor.matmul(
                                    ps[32 * j:32 * j + OUT, :],
                                    W2c[c2],
                                    sv[c2][:, 1024 * j + 512 * hf:
                                           1024 * j + 512 * (hf + 1)],
                                    start=(c2 == 0),
                                    stop=False,
                                    tile_position=(0, 32 * j),
                                )
                        for j in range(NBLK):
                            nc.tensor.matmul(
                                ps[32 * j:32 * j + OUT, :],
                                I10q_t[32 * j:32 * j + OUT, :],
                                ycur[32 * j:32 * j + OUT, sl],
                                start=False,
                                stop=False,
                                tile_position=(32 * j, 32 * j),
                            )
                        for j in range(NBLK):
                            nc.tensor.matmul(
                                ps[32 * j:32 * j + OUT, :],
                                I10d_t[32 * j:32 * j + OUT, :],
                                dbar[32 * j:32 * j + OUT, sl],
                                start=False,
                                stop=True,
                                tile_position=(32 * j, 32 * j),
                            )
                        yrt = yrp.tile([128, 512], f16, tag="yr", name="yr")
                        nc.scalar.activation(yrt[:], ps[:], Act.Relu)
                        nc.vector.tensor_scalar_min(ynext[:, sl], yrt[:], 1.0)

                    # ---- PE-merged chunks ----
                    for c in range(HCH):
                        if c == DVE_CHUNK:
                            continue
                        rch = rp.tile([128, BLOC], f16, tag="r", name="r")
                        for half in range(2):
                            pts = []
                            for rb in range(NBLK):
                                pu = pup.tile([128, 512], f32, tag="pu", name="pu")
                                nc.tensor.matmul(
                                    pu[:], halfI_t,
                                    sv[c][:, 1024 * rb + 512 * half:
                                          1024 * rb + 512 * (half + 1)],
                                    start=True, stop=False, tile_position=(0, 0),
                                )
                                pts.append((pu, rb))
                            for pu, rb in pts:
                                nc.tensor.matmul(
                                    pu[:],
                                    W2Tr[c][32 * rb:32 * rb + OUT, :],
                                    ycur[32 * rb:32 * rb + OUT,
                                         512 * half:512 * (half + 1)],
                                    start=False, stop=True,
                                    tile_position=(32 * rb, 0),
                                )
                            for pu, rb in pts:
                                nc.scalar.activation(
                                    rch[:, 1024 * rb + 512 * half:
                                        1024 * rb + 512 * (half + 1)],
                                    pu[:], Act.Relu,
                                )
                        finish_chunk(c, rch, on_gp=(c in GP_ADD_CHUNKS))

                    scur = snext
                    ycur = ynext

                # ---------- tail ----------
                for c in range(HCH):
                    nc.sync.dma_start(
                        out=hB_out[:, BLOC * c:BLOC * (c + 1)],
                        in_=scur[:, BLOC * c:BLOC * (c + 1)],
                    )
                nc.sync.dma_start(out=yB_out[:], in_=ycur[:])

    if not nc.is_finalized():
        nc.finalize()
    return nc


def _consts(W2, b2):
    cst16 = np.zeros((128, CF16_W), dtype=np.float16)
    cst16[:, C_HALFI:C_HALFI + 128] = 0.5 * np.eye(128, dtype=np.float16)
    W2h = 0.5 * W2.astype(np.float32)
    for c in range(HCH):
        blkT = W2h[128 * c:128 * (c + 1), :].T.astype(np.float16)  # [10, 128]
        for j in range(NBLK):
            cst16[32 * j:32 * j + OUT, C_W2TR + 128 * c:C_W2TR + 128 * (c + 1)] = blkT
        cst16[:, C_W2C + OUT * c:C_W2C + OUT * (c + 1)] = W2h[
            128 * c:128 * (c + 1), :
        ].astype(np.float16)
    for j in range(NBLK):
        for i in range(OUT):
            cst16[32 * j + i, C_I10Q + i] = 0.25
            cst16[32 * j + i, C_I10D + i] = 1.0
    return cst16


def kernel(**inputs):
    from concourse import bass_utils

    x = np.asarray(inputs["x"], dtype=np.float32)
    h0 = np.asarray(inputs["h_init"], dtype=np.float32)
    y0 = np.asarray(inputs["y_init"], dtype=np.float32)
    W1 = np.asarray(inputs["W1"], dtype=np.float32)
    W2 = np.asarray(inputs["W2"], dtype=np.float32)
    b1 = np.ascontiguousarray(
        np.asarray(inputs["b1"], dtype=np.float32).reshape(HID, 1)
    )
    b2 = np.asarray(inputs["b2"], dtype=np.float32).reshape(OUT)
    target = np.asarray(inputs["target"]).astype(np.int64)
    T = int(inputs["T"])

    xT16 = np.ascontiguousarray(x.T.astype(np.float16))      # [IN, B]
    hT16 = np.ascontiguousarray(h0.T.astype(np.float16))     # [HID, B]
    w1h = np.ascontiguousarray(
        W1.reshape(KIN, KC, HID).transpose(1, 0, 2).reshape(KC, KIN * HID)
    ).astype(np.float16)

    cst16 = _consts(W2, b2)

    key = T
    if key not in _BUILT:
        _BUILT[key] = _build(T)
    nc = _BUILT[key]

    in_maps = []
    for k in range(N_CORES):
        sl = slice(k * BLOC, (k + 1) * BLOC)
        tgt_k = target[sl]
        y0_k = y0[sl]  # [BLOC, OUT]
        yB = np.zeros((128, 1024), dtype=np.float16)
        dcon = np.zeros((128, 1024), dtype=np.float16)
        for j in range(NBLK):
            seg = slice(1024 * j, 1024 * (j + 1))
            yB[32 * j:32 * j + OUT, :] = y0_k[seg].T
            oh = (tgt_k[seg][None, :] == np.arange(OUT)[:, None])
            dcon[32 * j:32 * j + OUT, :] = (
                0.25 * oh + 0.5 * b2[:, None]
            ).astype(np.float16)
        ck = cst16.copy()
        ck[:, C_DCON:C_DCON + 1024] = dcon
        in_maps.append({
            "xT16": np.ascontiguousarray(xT16[:, sl]),
            "hT16": np.ascontiguousarray(hT16[:, sl]),
            "yB16": yB,
            "w1h": w1h,
            "b1": b1,
            "cst16": ck,
        })

    res = bass_utils.run_bass_kernel_spmd(nc, in_maps, list(range(N_CORES)))
    globals()["_LAST_RESULTS"] = res

    out = np.empty((B, HID + OUT), dtype=np.float32)
    for k in range(N_CORES):
        sl = slice(k * BLOC, (k + 1) * BLOC)
        hB = np.asarray(res.results[k]["hB_out"])  # [128, 4*4096] f16
        hT = hB.reshape(128, HCH, BLOC).transpose(1, 0, 2).reshape(HID, BLOC)
        out[sl, :HID] = hT.T.astype(np.float32)
        yB = np.asarray(res.results[k]["yB_out"])  # [128, 1024] f16
        yblk = yB.reshape(NBLK, 32, 1024)[:, :OUT, :]  # [4, 10, 1024]
        out[sl, HID:] = yblk.transpose(0, 2, 1).reshape(BLOC, OUT).astype(np.float32)
    return out
